# revision 1
# baseline (speedup 1.0000x reference)
"""Trainium2 Bass kernel for the 4-layer dense transformer (nn_BDH_GPU_65326452572468).

Sharding: 8 cores = 4 batches x 2 head-pairs. Core c handles batch c//2 and
heads {0,1} (c even) or {2,3} (c odd). Per layer, each core computes its two
heads' attention and dv contribution; dv is all-reduced within the core pair,
after which v stays replicated. Logits are taken from the even core of each pair.

All matmuls run in bf16 with fp32 PSUM accumulation; layernorm/softmax math is
fp32. Softmax uses a constant bias (no per-row max): scores for this model are
bounded (~12), diag >= 0, so exp(s - 16) neither overflows nor kills any row.
"""
import sys
import numpy as np

sys.path.insert(0, "/opt/trn_rl_repo")

import ml_dtypes

import concourse.bass as bass
import concourse.mybir as mybir
import concourse.tile as tile
from concourse import bacc
from concourse.bass_utils import run_bass_kernel_spmd

BF = ml_dtypes.bfloat16
FP32 = mybir.dt.float32
BF16 = mybir.dt.bfloat16
AL = mybir.AluOpType
AF = mybir.ActivationFunctionType
AX = mybir.AxisListType

D = 128
H = 4
L = 4
N = 4096
VOCAB = 256
DH = 32          # D // H
NH = 1024        # N // H
EPS = 1e-5
M_BIAS = 16.0    # constant softmax shift (max observed score ~12.2)
NCORES = 8
NCH = NH // 128  # 8 i-chunks per head



def _blob_offsets(T, apply_g1b1, apply_g2b2):
    """Word offsets (per 128-partition row) of each packed constant."""
    NT = T // 128
    offs, o = {}, 0
    def add(name, words):
        nonlocal o
        offs[name] = (o, words)
        o += words
    add("v0", NT * D // 2)            # bf16 [128, NT*D] (cast to fp32 on device)
    add("dxdy", NH // 2)              # bf16 [128, NH]: rows 0-63 dxl, 64-127 dyl
    add("encl", NCH * D)              # bf16 [128, 2*NCH*D]
    add("trig", 4 * (2 * (T // 64) + 128))  # fp32, per cp: sinA|cosA [128,T//64], sinB|cosB [128,64]
    add("mask", 128)                  # fp32 [128, 128]
    add("ident", 64)                  # bf16 [128, 128]
    add("rwt", VOCAB // 2)            # bf16 [128, VOCAB]
    add("sel", DH)                    # bf16 [128, 2*DH]
    if apply_g1b1:
        add("g1r", D); add("b1r", D)
    if apply_g2b2:
        add("g2r", D); add("b2r", D)
    offs["_total"] = o
    return offs

def build_kernel(T: int, apply_g1b1: bool, apply_g2b2: bool, use_collective: bool = True, n_layers: int = L):
    """Build the SPMD Bass program for sequence length T."""
    NT = T // 128          # t-tiles of 128
    NM = T // 512          # t-macros of 512
    nc = bacc.Bacc("TRN2", target_bir_lowering=False, debug=False,
                   num_devices=NCORES)

    # Single packed input [128, BW] fp32 (one IO tensor per direction: the axon
    # execution path pays ~2ms per bound tensor, so everything is packed).
    offs = _blob_offsets(T, apply_g1b1, apply_g2b2)
    blob_e = nc.dram_tensor("blob", [128, offs["_total"]], FP32,
                            kind="ExternalInput").ap()
    logits_e = nc.dram_tensor("logits", [T, VOCAB], FP32, kind="ExternalOutput").ap()

    def bslice(name, rows=128):
        o, w = offs[name]
        return blob_e[0:rows, o:o + w]

    from contextlib import ExitStack
    with tile.TileContext(nc) as tc, ExitStack() as stack:
        persist = stack.enter_context(tc.tile_pool(name="persist", bufs=1))
        work = stack.enter_context(tc.tile_pool(name="work", bufs=3))
        rowp = stack.enter_context(tc.tile_pool(name="rowp", bufs=2))
        smallp = stack.enter_context(tc.tile_pool(name="smallp", bufs=4))
        dramp = stack.enter_context(tc.tile_pool(name="dramp", bufs=2, space="DRAM"))
        ps_big = stack.enter_context(tc.tile_pool(name="ps_big", bufs=3, space="PSUM"))
        ps_tr = stack.enter_context(tc.tile_pool(name="ps_tr", bufs=2, space="PSUM"))
        ps_sm = stack.enter_context(tc.tile_pool(name="ps_sm", bufs=2, space="PSUM"))

        # ---- persistent SBUF state ----
        v_sb = persist.tile([128, NT * D], FP32)       # block t: v[t*128+p, d]
        dv_sb = persist.tile([128, NT * D], FP32)
        vn_sb = persist.tile([128, NT * D], BF16)
        vnT_sb = persist.tile([2 * DH, T], BF16)       # rows hl*32..: head hl vnT [d_local, t]
        vnh_sb = persist.tile([128, NT * 2 * DH], BF16)  # block s: vn[s-tokens, d_local 64]
        sel_sb = persist.tile([D, 2 * DH], BF16)
        x_sb = persist.tile([128, NCH * T], BF16)      # chunk c at free c*T
        xr_sb = persist.tile([128, NCH * T], BF16)     # rope'd x; reused as yi
        a_sb = persist.tile([2 * DH, T], BF16)
        cos_sb = persist.tile([128, 4 * T], BF16)
        sin_sb = persist.tile([128, 4 * T], BF16)
        dxl_sb = persist.tile([2 * DH, NH], BF16)
        dyl_sb = persist.tile([2 * DH, NH], BF16)
        encl_sb = persist.tile([128, 2 * NCH * D], BF16)
        mask_sb = persist.tile([128, 128], FP32)
        id_sb = persist.tile([128, 128], BF16)
        rwt_sb = persist.tile([D, VOCAB], BF16)
        mbias_sb = persist.tile([128, 1], FP32)
        eps_sb = persist.tile([128, 1], FP32)

        nc.sync.dma_start(vn_sb[:], bslice("v0").bitcast(BF16))
        nc.vector.tensor_copy(v_sb[:], vn_sb[:])
        nc.sync.dma_start(dxl_sb[:], bslice("dxdy", rows=64).bitcast(BF16))
        nc.sync.dma_start(dyl_sb[:], blob_e[64:128,
                          offs["dxdy"][0]:offs["dxdy"][0] + offs["dxdy"][1]].bitcast(BF16))
        nc.sync.dma_start(encl_sb[:], bslice("encl").bitcast(BF16))
        trig_sb = persist.tile([128, 4 * (2 * (T // 64) + 128)], FP32)
        nc.sync.dma_start(trig_sb[:], bslice("trig"))
        nc.sync.dma_start(mask_sb[:], bslice("mask"))
        nc.sync.dma_start(id_sb[:], bslice("ident").bitcast(BF16))
        nc.sync.dma_start(rwt_sb[:], bslice("rwt").bitcast(BF16))
        nc.sync.dma_start(sel_sb[:], bslice("sel").bitcast(BF16))
        nc.gpsimd.memset(mbias_sb[:], -M_BIAS)
        nc.gpsimd.memset(eps_sb[:], EPS)
        if apply_g1b1:
            g1_sb = persist.tile([128, D], FP32); nc.sync.dma_start(g1_sb[:], bslice("g1r"))
            b1_sb = persist.tile([128, D], FP32); nc.sync.dma_start(b1_sb[:], bslice("b1r"))
        if apply_g2b2:
            g2_sb = persist.tile([128, D], FP32); nc.sync.dma_start(g2_sb[:], bslice("g2r"))
            b2_sb = persist.tile([128, D], FP32); nc.sync.dma_start(b2_sb[:], bslice("b2r"))

        v3 = v_sb[:].rearrange("p (g d) -> p g d", d=D)       # [128, NT, D]
        vc_t = persist.tile([128, NT * D], FP32)              # ln centered scratch
        vc3 = vc_t[:].rearrange("p (g d) -> p g d", d=D)
        sq_t = persist.tile([128, NT * D], FP32)              # ln square scratch
        sq3 = sq_t[:].rearrange("p (g d) -> p g d", d=D)

        # Build rope sin/cos tables on device via sin(A+B)/cos(A+B) identities
        # from the small shipped tables (saves ~3.6MB/core of input staging).
        THI = T // 64
        CPW = 2 * THI + 128
        t1v = vc_t[:].rearrange("p (a b) -> p a b", b=64)     # [128, THI, 64] scratch
        t2v = sq_t[:].rearrange("p (a b) -> p a b", b=64)
        for cp in range(4):
            o = cp * CPW
            sA = trig_sb[:, o:o + THI, None].to_broadcast((128, THI, 64))
            cA = trig_sb[:, o + THI:o + 2 * THI, None].to_broadcast((128, THI, 64))
            sB = trig_sb[:, None, o + 2 * THI:o + 2 * THI + 64].to_broadcast((128, THI, 64))
            cB = trig_sb[:, None, o + 2 * THI + 64:o + CPW].to_broadcast((128, THI, 64))
            sin_o = sin_sb[:, cp * T:(cp + 1) * T].rearrange("p (a b) -> p a b", b=64)
            cos_o = cos_sb[:, cp * T:(cp + 1) * T].rearrange("p (a b) -> p a b", b=64)
            nc.vector.tensor_tensor(t1v, sA, cB, AL.mult)
            nc.vector.tensor_tensor(t2v, cA, sB, AL.mult)
            nc.vector.tensor_tensor(sin_o, t1v, t2v, AL.add)
            nc.vector.tensor_tensor(t1v, cA, cB, AL.mult)
            nc.vector.tensor_tensor(t2v, sA, sB, AL.mult)
            nc.vector.tensor_tensor(cos_o, t1v, t2v, AL.subtract)

        def layernorm_stats(src3):
            """Return (vc3 filled with src-mu, rs16 [128,NT])."""
            sums = smallp.tile([128, NT], FP32)
            nc.vector.reduce_sum(sums[:], src3, axis=AX.X)
            mu = smallp.tile([128, NT], FP32)
            nc.vector.tensor_scalar_mul(mu[:], sums[:], 1.0 / D)
            nc.vector.tensor_tensor(vc3, src3, mu[:, :, None].to_broadcast((128, NT, D)),
                                    AL.subtract)
            nc.vector.tensor_tensor(sq3, vc3, vc3, AL.mult)
            ssq = smallp.tile([128, NT], FP32)
            nc.vector.reduce_sum(ssq[:], sq3, axis=AX.X)
            var = smallp.tile([128, NT], FP32)
            nc.vector.tensor_scalar_mul(var[:], ssq[:], 1.0 / D)
            std = smallp.tile([128, NT], FP32)
            nc.scalar.activation(std[:], var[:], AF.Sqrt, bias=eps_sb[:], scale=1.0)
            rs = smallp.tile([128, NT], FP32)
            nc.vector.reciprocal(rs[:], std[:])
            return rs

        for l in range(n_layers):
            # ---------------- ln1 -> vn (bf16) ----------------
            rs = layernorm_stats(v3)
            vn3 = vn_sb[:].rearrange("p (g d) -> p g d", d=D)
            nc.vector.tensor_tensor(vn3, vc3, rs[:, :, None].to_broadcast((128, NT, D)),
                                    AL.mult)
            if apply_g1b1:
                nc.vector.tensor_tensor(vn3, vn3,
                                        g1_sb[:, None, :].to_broadcast((128, NT, D)), AL.mult)
                nc.vector.tensor_tensor(vn3, vn3,
                                        b1_sb[:, None, :].to_broadcast((128, NT, D)), AL.add)

            # vnT_full per tile; then select local head slices via sel matmuls
            # (per-core head choice lives in the `sel` input, keeping SPMD).
            for t in range(NT):
                ptr = ps_tr.tile([128, 128], BF16, tag="tr")
                nc.tensor.transpose(ptr[:], vn_sb[:, t * D:(t + 1) * D], id_sb[:])
                vtf = work.tile([128, 128], BF16, tag="vtf")
                nc.vector.tensor_copy(vtf[:], ptr[:])
                # vnT_sb[:, t-block] = sel.T @ vnT_full  -> [64, 128]
                p1 = ps_sm.tile([128, 128], FP32, tag="sm", name="p1")[:2 * DH, :]
                nc.tensor.matmul(p1[:], sel_sb[:], vtf[:], start=True, stop=True)
                nc.vector.tensor_copy(vnT_sb[:, t * 128:(t + 1) * 128], p1[:])
                # vnh_sb block t = vnT_full.T @ sel -> [128 tokens, 64]
                p2 = ps_sm.tile([128, 128], FP32, tag="sm", name="p2")[:, :2 * DH]
                nc.tensor.matmul(p2[:], vtf[:], sel_sb[:], start=True, stop=True)
                nc.vector.tensor_copy(vnh_sb[:, t * 2 * DH:(t + 1) * 2 * DH], p2[:])

            for hl in range(2):
                hr = slice(hl * DH, (hl + 1) * DH)       # rows in dxl/dyl/vnT
                # ---------------- X = relu(vr @ dx), chunk-major ----------------
                for m in range(NM):
                    tm = slice(m * 512, (m + 1) * 512)
                    for c in range(NCH):
                        px = ps_big.tile([128, 512], FP32, tag="big")
                        nc.tensor.matmul(px[:], dxl_sb[hr, c * 128:(c + 1) * 128],
                                         vnT_sb[hr, tm], start=True, stop=True)
                        nc.scalar.activation(x_sb[:, c * T + m * 512: c * T + (m + 1) * 512],
                                             px[:], AF.Relu)
                    # ---------------- rope for this t-macro ----------------
                    for cp in range(4):
                        xe = x_sb[:, cp * T + m * 512: cp * T + (m + 1) * 512]
                        xo = x_sb[:, (cp + 4) * T + m * 512: (cp + 4) * T + (m + 1) * 512]
                        co = cos_sb[:, cp * T + m * 512: cp * T + (m + 1) * 512]
                        si = sin_sb[:, cp * T + m * 512: cp * T + (m + 1) * 512]
                        re = xr_sb[:, cp * T + m * 512: cp * T + (m + 1) * 512]
                        ro = xr_sb[:, (cp + 4) * T + m * 512: (cp + 4) * T + (m + 1) * 512]
                        t1 = work.tile([128, 512], BF16, tag="rp1")
                        t2 = work.tile([128, 512], BF16, tag="rp2")
                        nc.vector.tensor_tensor(t1[:], xe, co, AL.mult)
                        nc.vector.tensor_tensor(t2[:], xo, si, AL.mult)
                        nc.vector.tensor_tensor(re, t1[:], t2[:], AL.subtract)
                        t3 = work.tile([128, 512], BF16, tag="rp1")
                        t4 = work.tile([128, 512], BF16, tag="rp2")
                        nc.vector.tensor_tensor(t3[:], xe, si, AL.mult)
                        nc.vector.tensor_tensor(t4[:], xo, co, AL.mult)
                        nc.vector.tensor_tensor(ro, t3[:], t4[:], AL.add)

                # ---------------- attention per t-tile ----------------
                for t in range(NT):
                    nblk = t + 1                      # causal s-blocks of 128
                    scols = nblk * 128
                    nsm = (scols + 511) // 512
                    prow = rowp.tile([128, NT * 128], BF16, tag="prow")
                    lparts = smallp.tile([128, 4], FP32, tag="lparts")
                    for sm in range(nsm):
                        w = min(512, scols - sm * 512)
                        pss = ps_big.tile([128, 512], FP32, tag="big")
                        for c in range(NCH):
                            nc.tensor.matmul(
                                pss[:, :w],
                                xr_sb[:, c * T + t * 128: c * T + (t + 1) * 128],
                                xr_sb[:, c * T + sm * 512: c * T + sm * 512 + w],
                                start=(c == 0), stop=(c == NCH - 1))
                        if sm == nsm - 1:
                            # diagonal 128-block mask (last 128 cols of this row)
                            nc.vector.tensor_tensor(pss[:, w - 128:w], pss[:, w - 128:w],
                                                    mask_sb[:], AL.add)
                        nc.scalar.activation(prow[:, sm * 512: sm * 512 + w], pss[:, :w],
                                             AF.Exp, bias=mbias_sb[:], scale=1.0,
                                             accum_out=lparts[:, sm:sm + 1])
                    lsum = smallp.tile([128, 1], FP32, tag="lsum")
                    if nsm > 1:
                        nc.vector.reduce_sum(lsum[:], lparts[:, :nsm], axis=AX.X)
                    else:
                        nc.vector.tensor_copy(lsum[:], lparts[:, 0:1])
                    rinv = smallp.tile([128, 1], FP32, tag="rinv")
                    nc.vector.reciprocal(rinv[:], lsum[:])
                    nc.vector.tensor_scalar_mul(prow[:, :scols], prow[:, :scols], rinv[:])
                    # transpose P blocks and accumulate a^T
                    pa = ps_sm.tile([128, 128], FP32, tag="sm", name="pa")[:2 * DH, :]
                    ptrow = rowp.tile([128, NT * 128], BF16, tag="ptrow")
                    for s in range(nblk):
                        ptr = ps_tr.tile([128, 128], BF16, tag="tr")
                        nc.tensor.transpose(ptr[:], prow[:, s * 128:(s + 1) * 128], id_sb[:])
                        nc.vector.tensor_copy(ptrow[:, s * 128:(s + 1) * 128], ptr[:])
                    for s in range(nblk):
                        nc.tensor.matmul(pa[:],
                                         vnh_sb[:, s * 2 * DH:(s + 1) * 2 * DH],
                                         ptrow[:, s * 128:(s + 1) * 128],
                                         start=(s == 0), stop=(s == nblk - 1))
                    nc.vector.tensor_copy(a_sb[hl * DH:(hl + 1) * DH, t * 128:(t + 1) * 128],
                                          pa[hl * DH:(hl + 1) * DH, :])

                # ---------------- YI = relu(a @ dy) * x -> xr_sb (reuse) ---------
                for m in range(NM):
                    tm = slice(m * 512, (m + 1) * 512)
                    for c in range(NCH):
                        py = ps_big.tile([128, 512], FP32, tag="big")
                        nc.tensor.matmul(py[:], dyl_sb[hr, c * 128:(c + 1) * 128],
                                         a_sb[hr, tm], start=True, stop=True)
                        rl = work.tile([128, 512], BF16, tag="rl")
                        nc.scalar.activation(rl[:], py[:], AF.Relu)
                        nc.vector.tensor_tensor(
                            xr_sb[:, c * T + m * 512: c * T + (m + 1) * 512], rl[:],
                            x_sb[:, c * T + m * 512: c * T + (m + 1) * 512], AL.mult)

                # ---------------- dv += yi @ enc ----------------
                for t in range(NT):
                    pd = ps_sm.tile([128, 128], FP32, tag="sm")
                    for c in range(NCH):
                        nc.tensor.matmul(
                            pd[:],
                            xr_sb[:, c * T + t * 128: c * T + (t + 1) * 128],
                            encl_sb[:, (hl * NCH + c) * D:(hl * NCH + c + 1) * D],
                            start=(c == 0), stop=(c == NCH - 1))
                    if hl == 0:
                        nc.vector.tensor_copy(dv_sb[:, t * D:(t + 1) * D], pd[:])
                    else:
                        nc.vector.tensor_tensor(dv_sb[:, t * D:(t + 1) * D],
                                                dv_sb[:, t * D:(t + 1) * D], pd[:], AL.add)

            # ---------------- pair all-reduce of dv; v += dv_tot -------------
            inb = dramp.tile([T, D], FP32, tag="inb")
            outb = dramp.tile([T, D], FP32, tag="outb")
            nc.gpsimd.dma_start(inb[:].rearrange("(g p) d -> p g d", p=128),
                                dv_sb[:].rearrange("p (g d) -> p g d", d=D))
            if use_collective:
                nc.gpsimd.collective_compute(
                    "AllReduce", AL.add,
                    replica_groups=[[0, 1], [2, 3], [4, 5], [6, 7]],
                    ins=[inb[:].opt()], outs=[outb[:].opt()])
            rb = outb if use_collective else inb
            nc.gpsimd.dma_start(dv_sb[:].rearrange("p (g d) -> p g d", d=D),
                                rb[:].rearrange("(g p) d -> p g d", p=128))
            nc.vector.tensor_tensor(v_sb[:], v_sb[:], dv_sb[:], AL.add)

            # ---------------- ln2: v = v + ln(v) ----------------
            rs2 = layernorm_stats(v3)
            nc.vector.tensor_tensor(vc3, vc3, rs2[:, :, None].to_broadcast((128, NT, D)),
                                    AL.mult)
            if apply_g2b2:
                nc.vector.tensor_tensor(vc3, vc3,
                                        g2_sb[:, None, :].to_broadcast((128, NT, D)), AL.mult)
                nc.vector.tensor_tensor(vc3, vc3,
                                        b2_sb[:, None, :].to_broadcast((128, NT, D)), AL.add)
            nc.vector.tensor_tensor(v_sb[:], v_sb[:], vc_t[:], AL.add)

        # ---------------- logits = v @ readout^T ----------------
        for t in range(NT):
            vb = work.tile([128, 128], BF16, tag="vb")
            nc.vector.tensor_copy(vb[:], v_sb[:, t * D:(t + 1) * D])
            ptr = ps_tr.tile([128, 128], BF16, tag="tr")
            nc.tensor.transpose(ptr[:], vb[:], id_sb[:])
            vtb = work.tile([128, 128], BF16, tag="vtb")
            nc.vector.tensor_copy(vtb[:], ptr[:])
            pl = ps_big.tile([128, 512], FP32, tag="big", name="pl")[:, :VOCAB]
            nc.tensor.matmul(pl[:], vtb[:], rwt_sb[:], start=True, stop=True)
            lf = work.tile([128, VOCAB], FP32, tag="lf")
            nc.vector.tensor_copy(lf[:], pl[:])
            nc.sync.dma_start(logits_e[t * 128:(t + 1) * 128, :], lf[:])


    nc.compile()
    return nc


# ---------------------------------------------------------------------------
# host-side preparation
# ---------------------------------------------------------------------------

def _prep_core_inputs(inputs, core, T):
    b = min(core // 2, np.asarray(inputs["idx"]).shape[0] - 1)
    heads = [0, 1] if core % 2 == 0 else [2, 3]

    idx = np.asarray(inputs["idx"])
    wte = np.asarray(inputs["wte"], np.float32)
    encoder = np.asarray(inputs["encoder"], np.float32)
    decoder_x = np.asarray(inputs["decoder_x"], np.float32)
    decoder_y = np.asarray(inputs["decoder_y"], np.float32)
    readout_w = np.asarray(inputs["readout_w"], np.float32)

    perm = np.concatenate([np.arange(0, NH, 2), np.arange(1, NH, 2)])

    v0 = wte[idx[b, :T]].astype(np.float32)                    # [T, D]

    dxl = np.concatenate([decoder_x[h][:, perm] for h in heads], 0).astype(BF)  # [64,1024]
    dyl = np.concatenate([decoder_y[h][:, perm] for h in heads], 0).astype(BF)

    encl = np.zeros((128, 2 * NCH * D), BF)
    encr = encoder.reshape(H, NH, D)
    for hl, h in enumerate(heads):
        ehp = encr[h][perm, :]                                  # [NH, D]
        for c in range(NCH):
            encl[:, (hl * NCH + c) * D:(hl * NCH + c + 1) * D] = \
                ehp[c * 128:(c + 1) * 128, :].astype(BF)

    div = np.exp(np.arange(0, NH, 2, dtype=np.float64) * (-np.log(10000.0) / NH))  # [512]
    THI = T // 64
    CPW = 2 * THI + 128
    trig = np.zeros((128, 4 * CPW), np.float32)
    thi = np.arange(THI, dtype=np.float64) * 64.0
    tlo = np.arange(64, dtype=np.float64)
    for cp in range(4):
        dk = div[cp * 128:(cp + 1) * 128][:, None]              # [128,1]
        o = cp * CPW
        trig[:, o:o + THI] = np.sin(dk * thi)
        trig[:, o + THI:o + 2 * THI] = np.cos(dk * thi)
        trig[:, o + 2 * THI:o + 2 * THI + 64] = np.sin(dk * tlo)
        trig[:, o + 2 * THI + 64:o + CPW] = np.cos(dk * tlo)

    mask = np.triu(np.full((128, 128), -1e30, np.float32), 1)
    ident = np.eye(128, dtype=np.float32).astype(BF)
    rwt = readout_w.T.astype(BF)                                # [128, 256]
    sel = np.zeros((D, 2 * DH), np.float32)
    for j, h in enumerate(heads):
        sel[h * DH:(h + 1) * DH, j * DH:(j + 1) * DH] = np.eye(DH)
    sel = sel.astype(BF)

    g1 = np.asarray(inputs["ln1_g"], np.float32); b1 = np.asarray(inputs["ln1_b"], np.float32)
    g2 = np.asarray(inputs["ln2_g"], np.float32); b2 = np.asarray(inputs["ln2_b"], np.float32)
    a1 = not (np.all(g1 == 1.0) and np.all(b1 == 0.0))
    a2 = not (np.all(g2 == 1.0) and np.all(b2 == 0.0))

    offs = _blob_offsets(T, a1, a2)
    blob = np.zeros((128, offs["_total"]), np.float32)

    def put32(name, arr, rows=slice(0, 128)):
        o, w = offs[name]
        blob[rows, o:o + w] = arr
    def putbf(name, arr_bf, rows=slice(0, 128)):
        o, w = offs[name]
        blob[rows, o:o + arr_bf.shape[1] // 2] =             np.ascontiguousarray(arr_bf).view(np.float32)

    NT = T // 128
    putbf("v0", v0.reshape(NT, 128, D).transpose(1, 0, 2).reshape(128, NT * D).astype(BF))
    putbf("dxdy", dxl, rows=slice(0, 64))
    putbf("dxdy", dyl, rows=slice(64, 128))
    putbf("encl", encl)
    put32("trig", trig)
    put32("mask", mask)
    putbf("ident", ident)
    putbf("rwt", rwt)
    putbf("sel", sel)
    if a1:
        put32("g1r", np.broadcast_to(g1, (128, D)))
        put32("b1r", np.broadcast_to(b1, (128, D)))
    if a2:
        put32("g2r", np.broadcast_to(g2, (128, D)))
        put32("b2r", np.broadcast_to(b2, (128, D)))
    return {"blob": blob}


_BUILT = {}


def _get_kernel(T, apply_g1b1, apply_g2b2):
    key = (T, apply_g1b1, apply_g2b2)
    if key not in _BUILT:
        _BUILT[key] = build_kernel(T, apply_g1b1, apply_g2b2)
    return _BUILT[key]


def kernel(**inputs) -> np.ndarray:
    idx = np.asarray(inputs["idx"])
    B, T = idx.shape
    g1 = np.asarray(inputs["ln1_g"], np.float32); b1 = np.asarray(inputs["ln1_b"], np.float32)
    g2 = np.asarray(inputs["ln2_g"], np.float32); b2 = np.asarray(inputs["ln2_b"], np.float32)
    a1 = not (np.all(g1 == 1.0) and np.all(b1 == 0.0))
    a2 = not (np.all(g2 == 1.0) and np.all(b2 == 0.0))

    nc = _get_kernel(T, a1, a2)
    in_maps = [_prep_core_inputs(inputs, c, T) for c in range(NCORES)]
    res = run_bass_kernel_spmd(nc, in_maps, list(range(NCORES)))
    out = np.stack([res.results[2 * b]["logits"] for b in range(B)], 0)
    return out.astype(np.float32)



# revision 9
# speedup vs baseline: 1.8019x; 1.8019x over previous
"""Trainium2 Bass kernel for the 4-layer dense transformer (nn_BDH_GPU_65326452572468).

Sharding: 8 cores = 4 batches x 2 head-pairs. Core c handles batch c//2 and
heads {0,1} (c even) or {2,3} (c odd). Per layer, each core computes its two
heads' attention and dv contribution; dv is all-reduced within the core pair,
after which v stays replicated. Logits are taken from the even core of each pair.

v2 design notes (vs the v1 baseline):
- v lives TRANSPOSED on device: vT [d=128 partitions, T free]. LN stats are
  computed with ones-weight matmuls over the partition (d) axis; the per-token
  affine is applied via gpsimd partition_broadcast + two DVE ops.
- Attention computes scores TRANSPOSED (S^T[s,t] blocks) by swapping matmul
  operands, so P^T is produced directly and the ~1100 PE transposes of the v1
  kernel disappear. Softmax row-sums come free from a ones-column appended to
  the V operand (vnh_aug [s, 65]); normalization scales a^T per-column after
  accumulation (relu commutes with the positive scale).
- Causal masking: diagonal s-blocks stream only their valid t-range
  (N in {512,384,256,128}) and the one triangular 128-wide subtile is masked
  by multiplying exp by a 0/1 triu mask (bf16, cheap).
- x = relu(vn@dx) / yi-gemms use tile_position row-packing (K=32 -> 4-way /
  2-way concurrent in the 128x128 PE array).
- All matmuls bf16 with fp32 PSUM; softmax uses the constant-bias trick
  (scores bounded ~12.2, bias 16).
"""
import sys
import numpy as np

sys.path.insert(0, "/opt/trn_rl_repo")

import ml_dtypes

import concourse.bass as bass
import concourse.mybir as mybir
import concourse.tile as tile
from concourse import bacc
from concourse.bass_utils import run_bass_kernel_spmd

BF = ml_dtypes.bfloat16
FP32 = mybir.dt.float32
BF16 = mybir.dt.bfloat16
AL = mybir.AluOpType
AF = mybir.ActivationFunctionType
AX = mybir.AxisListType

D = 128
H = 4
L = 4
N = 4096
VOCAB = 256
DH = 32          # D // H
NH = 1024        # N // H
EPS = 1e-5
M_BIAS = 16.0    # constant softmax shift (max observed score ~12.2)
NCORES = 8
NCH = NH // 128  # 8 i-chunks per head
RSQRT_D = float(1.0 / np.sqrt(128.0))


def _blob_offsets(T, apply_g1b1, apply_g2b2):
    """Word offsets (per 128-partition row) of each packed constant."""
    offs, o = {}, 0
    def add(name, words):
        nonlocal o
        offs[name] = (o, words)
        o += words
    add("v0T", T // 2)               # bf16 [128, T]  (d-major v0)
    add("dxl2", NH // 2)             # bf16 [128, NH]: h0,h1,h0,h1 in 32-row groups
    add("dyl", NH // 2)              # bf16 rows 32-63 h0, 64-95 h1
    add("encl", NCH * D)             # bf16 [128, 2*NCH*D]
    add("trig", 4 * (2 * (T // 64) + 128))  # fp32 rope seed tables
    add("triu", 64)                  # bf16 [128,128] 0/1, keep t>=s
    add("ident", 64)                 # bf16 [128,128]
    add("rwt", VOCAB // 2)           # bf16 [128, VOCAB]
    add("sel", 64)                   # bf16 [128,128] d -> compact-row selection
    if apply_g1b1:
        add("g1c", 1); add("b1c", 1)
    if apply_g2b2:
        add("g2f", 1); add("b2f", 1)
    offs["_total"] = o
    return offs


def build_kernel(T: int, apply_g1b1: bool, apply_g2b2: bool,
                 use_collective: bool = True, n_layers: int = L):
    NT = T // 128
    NM = T // 512
    nc = bacc.Bacc("TRN2", target_bir_lowering=False, debug=False,
                   num_devices=NCORES)

    offs = _blob_offsets(T, apply_g1b1, apply_g2b2)
    blob_e = nc.dram_tensor("blob", [128, offs["_total"]], FP32,
                            kind="ExternalInput").ap()
    logits_e = nc.dram_tensor("logits", [T, VOCAB], FP32, kind="ExternalOutput").ap()

    def bslice(name, rows=128):
        o, w = offs[name]
        return blob_e[0:rows, o:o + w]

    from contextlib import ExitStack
    with tile.TileContext(nc) as tc, ExitStack() as stack:
        persist = stack.enter_context(tc.tile_pool(name="persist", bufs=1))
        work = stack.enter_context(tc.tile_pool(name="work", bufs=3))
        ropet = stack.enter_context(tc.tile_pool(name="ropet", bufs=2))
        stat = stack.enter_context(tc.tile_pool(name="stat", bufs=2))
        dramp = stack.enter_context(tc.tile_pool(name="dramp", bufs=2, space="DRAM"))
        ps_s = stack.enter_context(tc.tile_pool(name="ps_s", bufs=2, space="PSUM"))
        ps_a = stack.enter_context(tc.tile_pool(name="ps_a", bufs=2, space="PSUM"))
        ps_w = stack.enter_context(tc.tile_pool(name="ps_w", bufs=3, space="PSUM"))
        ps_t = stack.enter_context(tc.tile_pool(name="ps_t", bufs=1, space="PSUM"))

        # ---- persistent SBUF state ----
        vT = persist.tile([128, T], FP32)             # v transposed [d, t]
        dv_sb = persist.tile([128, T], FP32)          # dv^T (also rope scratch)
        dv2_sb = persist.tile([128, T], FP32)         # reduced dv^T (also rope scratch)
        vb_sb = persist.tile([128, T], BF16)          # bf16 cast of vT
        sq_sb = persist.tile([128, T], BF16)          # (vT^2)/128 bf16
        vnTc = persist.tile([128, T], BF16)           # compact ln1(v): h0,h1,h0,h1
        vnh = persist.tile([128, NT * 96], BF16)      # [s,96]: col0=ones, 32-95=vn
        aT_sb = persist.tile([96, T], BF16)           # a^T rows 32-63 h0, 64-95 h1
        x0_sb = persist.tile([128, NCH * T], BF16)    # x head0, chunk c at c*T
        x1_sb = persist.tile([128, NCH * T], BF16)
        xr0_sb = persist.tile([128, NCH * T], BF16)   # rope(x) head0; reused as yi0
        xr1_sb = persist.tile([128, NCH * T], BF16)
        cos_sb = persist.tile([128, 4 * T], BF16)
        sin_sb = persist.tile([128, 4 * T], BF16)
        dxl2_sb = persist.tile([128, NH], BF16)
        dyl_sb = persist.tile([96, NH], BF16)         # rows 32-63 h0, 64-95 h1
        encl_sb = persist.tile([128, 2 * NCH * D], BF16)
        sel_sb = persist.tile([128, 128], BF16)
        triu_sb = persist.tile([128, 128], BF16)
        id_sb = persist.tile([128, 128], BF16)
        rwt_sb = persist.tile([D, VOCAB], BF16)
        mbias_sb = persist.tile([128, 1], FP32)
        eps_sb = persist.tile([128, 1], FP32)
        onesA = persist.tile([128, 1], BF16)          # 1/128
        onesB = persist.tile([128, 1], BF16)          # 1.0

        nc.sync.dma_start(vb_sb[:], bslice("v0T").bitcast(BF16))
        nc.vector.tensor_copy(vT[:], vb_sb[:])
        nc.sync.dma_start(dxl2_sb[:], bslice("dxl2").bitcast(BF16))
        nc.sync.dma_start(dyl_sb[:], bslice("dyl", rows=96).bitcast(BF16))
        nc.sync.dma_start(encl_sb[:], bslice("encl").bitcast(BF16))
        trig_sb = persist.tile([128, 4 * (2 * (T // 64) + 128)], FP32)
        nc.sync.dma_start(trig_sb[:], bslice("trig"))
        nc.sync.dma_start(triu_sb[:], bslice("triu").bitcast(BF16))
        nc.sync.dma_start(id_sb[:], bslice("ident").bitcast(BF16))
        nc.sync.dma_start(rwt_sb[:], bslice("rwt").bitcast(BF16))
        nc.sync.dma_start(sel_sb[:], bslice("sel").bitcast(BF16))
        nc.gpsimd.memset(mbias_sb[:], -M_BIAS)
        nc.gpsimd.memset(eps_sb[:], EPS)
        nc.gpsimd.memset(onesA[:], 1.0 / 128.0)
        nc.gpsimd.memset(onesB[:], 1.0)
        vnh3 = vnh[:].rearrange("p (g c) -> p g c", c=96)
        nc.gpsimd.memset(vnh3[:, :, 0:1], 1.0)
        if apply_g1b1:
            g1c_sb = persist.tile([128, 1], FP32); nc.sync.dma_start(g1c_sb[:], bslice("g1c"))
            b1c_sb = persist.tile([128, 1], FP32); nc.sync.dma_start(b1c_sb[:], bslice("b1c"))
        if apply_g2b2:
            g2f_sb = persist.tile([128, 1], FP32); nc.sync.dma_start(g2f_sb[:], bslice("g2f"))
            b2f_sb = persist.tile([128, 1], FP32); nc.sync.dma_start(b2f_sb[:], bslice("b2f"))

        # Build rope sin/cos tables on device via sin(A+B)/cos(A+B) identities.
        THI = T // 64
        CPW = 2 * THI + 128
        t1v = dv_sb[:].rearrange("p (a b) -> p a b", b=64)
        t2v = dv2_sb[:].rearrange("p (a b) -> p a b", b=64)
        for cp in range(4):
            o = cp * CPW
            sA = trig_sb[:, o:o + THI, None].to_broadcast((128, THI, 64))
            cA = trig_sb[:, o + THI:o + 2 * THI, None].to_broadcast((128, THI, 64))
            sB = trig_sb[:, None, o + 2 * THI:o + 2 * THI + 64].to_broadcast((128, THI, 64))
            cB = trig_sb[:, None, o + 2 * THI + 64:o + CPW].to_broadcast((128, THI, 64))
            sin_o = sin_sb[:, cp * T:(cp + 1) * T].rearrange("p (a b) -> p a b", b=64)
            cos_o = cos_sb[:, cp * T:(cp + 1) * T].rearrange("p (a b) -> p a b", b=64)
            nc.vector.tensor_tensor(t1v, sA, cB, AL.mult)
            nc.vector.tensor_tensor(t2v, cA, sB, AL.mult)
            nc.vector.tensor_tensor(sin_o, t1v, t2v, AL.add)
            nc.vector.tensor_tensor(t1v, cA, cB, AL.mult)
            nc.vector.tensor_tensor(t2v, sA, sB, AL.mult)
            nc.vector.tensor_tensor(cos_o, t1v, t2v, AL.subtract)

        def ln_pass(m):
            """Per-macro LN stats on vT; returns (rs_b, mr_b) broadcast tiles."""
            tm = slice(m * 512, (m + 1) * 512)
            nc.vector.tensor_copy(vb_sb[:, tm], vT[:, tm])
            nc.scalar.activation(sq_sb[:, tm], vT[:, tm], AF.Square, scale=RSQRT_D)
            mu_ps = ps_w.tile([1, 512], FP32, tag="w", name="mu")
            nc.tensor.matmul(mu_ps[:], onesA[:], vb_sb[:, tm], start=True, stop=True)
            m2_ps = ps_w.tile([1, 512], FP32, tag="w", name="m2")
            nc.tensor.matmul(m2_ps[:], onesB[:], sq_sb[:, tm], start=True, stop=True)
            msq = stat.tile([1, 512], FP32, tag="st1")
            nc.scalar.activation(msq[:], mu_ps[:], AF.Square)
            var = stat.tile([1, 512], FP32, tag="st1")
            nc.vector.tensor_tensor(var[:], m2_ps[:], msq[:], AL.subtract)
            lnv = stat.tile([1, 512], FP32, tag="st1")
            nc.scalar.activation(lnv[:], var[:], AF.Ln, bias=eps_sb[0:1, :], scale=1.0)
            rs = stat.tile([1, 512], FP32, tag="st2")
            nc.scalar.activation(rs[:], lnv[:], AF.Exp, scale=-0.5)
            mr = stat.tile([1, 512], FP32, tag="st2")
            nc.vector.tensor_tensor(mr[:], mu_ps[:], rs[:], AL.mult)
            rs_b = stat.tile([128, 512], FP32, tag="stb")
            nc.gpsimd.partition_broadcast(rs_b[:], rs[:], channels=128)
            mr_b = stat.tile([128, 512], FP32, tag="stb")
            nc.gpsimd.partition_broadcast(mr_b[:], mr[:], channels=128)
            return rs_b, mr_b

        for l in range(n_layers):
            # ---------------- ln1 -> vnTc (compact, bf16) + vnh ----------------
            for m in range(NM):
                tm = slice(m * 512, (m + 1) * 512)
                rs_b, mr_b = ln_pass(m)
                selv = ps_w.tile([128, 512], FP32, tag="w", name="selv")
                nc.tensor.matmul(selv[:], sel_sb[:], vb_sb[:, tm], start=True, stop=True)
                tmp = work.tile([128, 512], FP32, tag="lnt")
                nc.vector.tensor_tensor(tmp[:], selv[:], rs_b[:], AL.mult)
                nc.vector.tensor_tensor(vnTc[:, tm], tmp[:], mr_b[:], AL.subtract)
                if apply_g1b1:
                    nc.vector.tensor_scalar_mul(vnTc[:, tm], vnTc[:, tm], g1c_sb[:])
                    nc.vector.tensor_scalar(vnTc[:, tm], vnTc[:, tm], b1c_sb[:], AL.add)
                for q in range(4):
                    tt = 4 * m + q
                    ptr = ps_t.tile([128, 64], BF16, tag="tr")
                    nc.tensor.transpose(ptr[:], vnTc[0:64, tt * 128:(tt + 1) * 128],
                                        id_sb[0:64, 0:64])
                    nc.vector.tensor_copy(vnh3[:, tt, 32:96], ptr[:])

            # ---------------- X = relu(vn @ dx), 4-way row-packed ----------------
            # chunk pairs (c, c+4) so rope chunk-pair cp completes early
            for cp in range(4):
                ca, cb = cp, cp + 4
                for m in range(NM):
                    tm = slice(m * 512, (m + 1) * 512)
                    for (rg, cc, xout) in ((0, ca, x0_sb), (32, ca, x1_sb),
                                           (64, cb, x0_sb), (96, cb, x1_sb)):
                        px = ps_w.tile([128, 512], FP32, tag="w", name="px")
                        nc.tensor.matmul(px[:], dxl2_sb[rg:rg + 32, cc * 128:(cc + 1) * 128],
                                         vnTc[rg:rg + 32, tm], start=True, stop=True,
                                         tile_position=(rg, 0))
                        nc.scalar.activation(
                            xout[:, cc * T + m * 512: cc * T + (m + 1) * 512],
                            px[:], AF.Relu)
                # ---------------- rope for this chunk pair, both heads -----------
                for x_s, xr_s in ((x0_sb, xr0_sb), (x1_sb, xr1_sb)):
                    xe = x_s[:, ca * T:(ca + 1) * T]
                    xo = x_s[:, cb * T:(cb + 1) * T]
                    co = cos_sb[:, cp * T:(cp + 1) * T]
                    si = sin_sb[:, cp * T:(cp + 1) * T]
                    t1 = ropet.tile([128, T], BF16, tag="r1")
                    t2 = ropet.tile([128, T], BF16, tag="r2")
                    nc.vector.tensor_tensor(t1[:], xe, co, AL.mult)
                    nc.vector.tensor_tensor(t2[:], xo, si, AL.mult)
                    nc.vector.tensor_tensor(xr_s[:, ca * T:(ca + 1) * T], t1[:], t2[:],
                                            AL.subtract)
                    t3 = ropet.tile([128, T], BF16, tag="r1")
                    t4 = ropet.tile([128, T], BF16, tag="r2")
                    nc.vector.tensor_tensor(t3[:], xe, si, AL.mult)
                    nc.vector.tensor_tensor(t4[:], xo, co, AL.mult)
                    nc.vector.tensor_tensor(xr_s[:, cb * T:(cb + 1) * T], t3[:], t4[:],
                                            AL.add)

            # ---------------- attention (transposed scores) ----------------
            for hl, xr_s in enumerate((xr0_sb, xr1_sb)):
                for m in range(NM):
                    aT_ps = ps_a.tile([96, 512], FP32, tag="a")
                    nblk = 4 * m + 4
                    for k in range(nblk):
                        j = k - 4 * m
                        if j < 0:
                            toff, w = m * 512, 512
                        else:
                            toff, w = m * 512 + 128 * j, 512 - 128 * j
                        pss = ps_s.tile([128, 512], FP32, tag="s")
                        for c in range(NCH):
                            nc.tensor.matmul(
                                pss[:, :w],
                                xr_s[:, c * T + 128 * k: c * T + 128 * (k + 1)],
                                xr_s[:, c * T + toff: c * T + toff + w],
                                start=(c == 0), stop=(c == NCH - 1))
                        ex = work.tile([128, 512], BF16, tag="ex")
                        nc.scalar.activation(ex[:, :w], pss[:, :w], AF.Exp,
                                             bias=mbias_sb[:], scale=1.0)
                        if j >= 0:
                            nc.vector.tensor_tensor(ex[:, 0:128], ex[:, 0:128],
                                                    triu_sb[:], AL.mult)
                        nc.tensor.matmul(aT_ps[:, toff - m * 512: toff - m * 512 + w],
                                         vnh3[:, k, 0:96], ex[:, :w],
                                         start=(k == 0), stop=(k == nblk - 1),
                                         skip_group_check=True)
                    # normalize this head's rows by 1/rowsum = exp(-ln(rowsum))
                    r1 = stat.tile([1, 512], FP32, tag="rq")
                    nc.scalar.activation(r1[:], aT_ps[0:1, :], AF.Ln)
                    r2 = stat.tile([1, 512], FP32, tag="rq")
                    nc.scalar.activation(r2[:], r1[:], AF.Exp, scale=-1.0)
                    rinv_b = stat.tile([96, 512], FP32, tag="stb")
                    nc.gpsimd.partition_broadcast(rinv_b[:], r2[:], channels=96)
                    rr = slice(32 + 32 * hl, 64 + 32 * hl)
                    nc.vector.tensor_tensor(
                        aT_sb[rr, m * 512:(m + 1) * 512],
                        aT_ps[rr, :], rinv_b[rr, :], AL.mult)

            # ---------------- YI = relu(a @ dy) * x ; dv^T += enc^T @ yi ---------
            for m in range(NM):
                tm = slice(m * 512, (m + 1) * 512)
                dvp = ps_w.tile([128, 512], FP32, tag="w", name="dvp")
                nmm = 0
                for c in range(NCH):
                    for hl, (x_s, yi_s) in enumerate(((x0_sb, xr0_sb), (x1_sb, xr1_sb))):
                        py = ps_w.tile([128, 512], FP32, tag="w", name="py")
                        nc.tensor.matmul(py[:],
                                         dyl_sb[32 + 32 * hl:64 + 32 * hl, c * 128:(c + 1) * 128],
                                         aT_sb[32 + 32 * hl:64 + 32 * hl, tm],
                                         start=True, stop=True)
                        rl = work.tile([128, 512], BF16, tag="rl")
                        nc.scalar.activation(rl[:], py[:], AF.Relu)
                        nc.vector.tensor_tensor(
                            yi_s[:, c * T + m * 512: c * T + (m + 1) * 512], rl[:],
                            x_s[:, c * T + m * 512: c * T + (m + 1) * 512], AL.mult)
                        nc.tensor.matmul(
                            dvp[:],
                            encl_sb[:, (hl * NCH + c) * D:(hl * NCH + c + 1) * D],
                            yi_s[:, c * T + m * 512: c * T + (m + 1) * 512],
                            start=(nmm == 0), stop=(nmm == 2 * NCH - 1),
                            skip_group_check=True)
                        nmm += 1
                nc.vector.tensor_copy(dv_sb[:, tm], dvp[:])

            # ---------------- pair all-reduce of dv; v += dv_tot -------------
            inb = dramp.tile([128, T], FP32, tag="inb")
            outb = dramp.tile([128, T], FP32, tag="outb")
            nc.gpsimd.dma_start(inb[:], dv_sb[:])
            if use_collective:
                nc.gpsimd.collective_compute(
                    "AllReduce", AL.add,
                    replica_groups=[[0, 1], [2, 3], [4, 5], [6, 7]],
                    ins=[inb[:].opt()], outs=[outb[:].opt()])
            rb = outb if use_collective else inb
            nc.gpsimd.dma_start(dv2_sb[:], rb[:])

            # ---------------- v += dv; ln2: v = v + ln(v) ----------------
            for m in range(NM):
                tm = slice(m * 512, (m + 1) * 512)
                nc.vector.tensor_tensor(vT[:, tm], vT[:, tm], dv2_sb[:, tm], AL.add)
                rs_b, mr_b = ln_pass(m)
                t0 = work.tile([128, 512], FP32, tag="lnt")
                nc.vector.tensor_tensor(t0[:], vT[:, tm], rs_b[:], AL.mult)
                t1 = work.tile([128, 512], FP32, tag="lnt2")
                nc.vector.tensor_tensor(t1[:], t0[:], mr_b[:], AL.subtract)
                if apply_g2b2:
                    nc.vector.tensor_scalar_mul(t1[:], t1[:], g2f_sb[:])
                    nc.vector.tensor_scalar(t1[:], t1[:], b2f_sb[:], AL.add)
                nc.vector.tensor_tensor(vT[:, tm], vT[:, tm], t1[:], AL.add)

        # ---------------- logits = v @ readout^T ----------------
        for m in range(NM):
            tm = slice(m * 512, (m + 1) * 512)
            nc.vector.tensor_copy(vb_sb[:, tm], vT[:, tm])
        for tt in range(NT):
            pl = ps_w.tile([128, VOCAB], FP32, tag="w", name="pl")
            nc.tensor.matmul(pl[:], vb_sb[:, tt * 128:(tt + 1) * 128], rwt_sb[:],
                             start=True, stop=True)
            lf = work.tile([128, VOCAB], FP32, tag="lf")
            nc.vector.tensor_copy(lf[:], pl[:])
            nc.sync.dma_start(logits_e[tt * 128:(tt + 1) * 128, :], lf[:])

    nc.compile()
    return nc


# ---------------------------------------------------------------------------
# host-side preparation
# ---------------------------------------------------------------------------

def _prep_core_inputs(inputs, core, T):
    b = min(core // 2, np.asarray(inputs["idx"]).shape[0] - 1)
    heads = [0, 1] if core % 2 == 0 else [2, 3]

    idx = np.asarray(inputs["idx"])
    wte = np.asarray(inputs["wte"], np.float32)
    encoder = np.asarray(inputs["encoder"], np.float32)
    decoder_x = np.asarray(inputs["decoder_x"], np.float32)
    decoder_y = np.asarray(inputs["decoder_y"], np.float32)
    readout_w = np.asarray(inputs["readout_w"], np.float32)

    perm = np.concatenate([np.arange(0, NH, 2), np.arange(1, NH, 2)])

    v0T = wte[idx[b, :T]].astype(np.float32).T                 # [D, T]

    dxh = [decoder_x[h][:, perm].astype(BF) for h in heads]    # [32,1024] each
    dyh = [decoder_y[h][:, perm].astype(BF) for h in heads]
    dxl2 = np.concatenate([dxh[0], dxh[1], dxh[0], dxh[1]], 0)  # [128,1024]
    dyl = np.concatenate([np.zeros((32, NH), BF), dyh[0], dyh[1]], 0)  # [96,1024]

    encl = np.zeros((128, 2 * NCH * D), BF)
    encr = encoder.reshape(H, NH, D)
    for hl, h in enumerate(heads):
        ehp = encr[h][perm, :]                                  # [NH, D]
        for c in range(NCH):
            encl[:, (hl * NCH + c) * D:(hl * NCH + c + 1) * D] = \
                ehp[c * 128:(c + 1) * 128, :].astype(BF)

    div = np.exp(np.arange(0, NH, 2, dtype=np.float64) * (-np.log(10000.0) / NH))
    THI = T // 64
    CPW = 2 * THI + 128
    trig = np.zeros((128, 4 * CPW), np.float32)
    thi = np.arange(THI, dtype=np.float64) * 64.0
    tlo = np.arange(64, dtype=np.float64)
    for cp in range(4):
        dk = div[cp * 128:(cp + 1) * 128][:, None]              # [128,1]
        o = cp * CPW
        trig[:, o:o + THI] = np.sin(dk * thi)
        trig[:, o + THI:o + 2 * THI] = np.cos(dk * thi)
        trig[:, o + 2 * THI:o + 2 * THI + 64] = np.sin(dk * tlo)
        trig[:, o + 2 * THI + 64:o + CPW] = np.cos(dk * tlo)

    triu = np.triu(np.ones((128, 128), np.float32), 0).astype(BF)  # keep t>=s
    ident = np.eye(128, dtype=np.float32).astype(BF)
    rwt = readout_w.T.astype(BF)                                # [128, 256]
    sel = np.zeros((128, 128), np.float32)
    for j in range(128):
        hl = (j // 32) % 2
        sel[heads[hl] * DH + (j % 32), j] = 1.0
    sel = sel.astype(BF)

    g1 = np.asarray(inputs["ln1_g"], np.float32); b1 = np.asarray(inputs["ln1_b"], np.float32)
    g2 = np.asarray(inputs["ln2_g"], np.float32); b2 = np.asarray(inputs["ln2_b"], np.float32)
    a1 = not (np.all(g1 == 1.0) and np.all(b1 == 0.0))
    a2 = not (np.all(g2 == 1.0) and np.all(b2 == 0.0))

    offs = _blob_offsets(T, a1, a2)
    blob = np.zeros((128, offs["_total"]), np.float32)

    def put32(name, arr, rows=slice(0, 128)):
        o, w = offs[name]
        blob[rows, o:o + w] = arr
    def putbf(name, arr_bf, rows=slice(0, 128)):
        o, w = offs[name]
        blob[rows, o:o + arr_bf.shape[1] // 2] = \
            np.ascontiguousarray(arr_bf).view(np.float32)

    putbf("v0T", v0T.astype(BF))
    putbf("dxl2", dxl2)
    putbf("dyl", dyl, rows=slice(0, 96))
    putbf("encl", encl)
    put32("trig", trig)
    putbf("triu", triu)
    putbf("ident", ident)
    putbf("rwt", rwt)
    putbf("sel", sel)
    if a1:
        g1c = np.array([g1[heads[(j // 32) % 2] * DH + (j % 32)] for j in range(128)])
        b1c = np.array([b1[heads[(j // 32) % 2] * DH + (j % 32)] for j in range(128)])
        put32("g1c", g1c[:, None])
        put32("b1c", b1c[:, None])
    if a2:
        put32("g2f", g2[:, None])
        put32("b2f", b2[:, None])
    return {"blob": blob}


_BUILT = {}


def _get_kernel(T, apply_g1b1, apply_g2b2):
    key = (T, apply_g1b1, apply_g2b2)
    if key not in _BUILT:
        _BUILT[key] = build_kernel(T, apply_g1b1, apply_g2b2)
    return _BUILT[key]


def kernel(**inputs) -> np.ndarray:
    idx = np.asarray(inputs["idx"])
    B, T = idx.shape
    g1 = np.asarray(inputs["ln1_g"], np.float32); b1 = np.asarray(inputs["ln1_b"], np.float32)
    g2 = np.asarray(inputs["ln2_g"], np.float32); b2 = np.asarray(inputs["ln2_b"], np.float32)
    a1 = not (np.all(g1 == 1.0) and np.all(b1 == 0.0))
    a2 = not (np.all(g2 == 1.0) and np.all(b2 == 0.0))

    nc = _get_kernel(T, a1, a2)
    in_maps = [_prep_core_inputs(inputs, c, T) for c in range(NCORES)]
    res = run_bass_kernel_spmd(nc, in_maps, list(range(NCORES)))
    out = np.stack([res.results[2 * b]["logits"] for b in range(B)], 0)
    return out.astype(np.float32)
